# revision 36
# baseline (speedup 1.0000x reference)
"""Sparse-attention Trainium2 kernel (nn_AttentionLayer, B=16 S=2048 D=128).

reference semantics:
    A = Q @ T^T                     # [B,S,S]
    A = where(A > 0.3, A, 0)
    A += where(strictly_upper, -2^32, 0)
    y = softmax(A / sqrt(D)) @ V

Sharding: data-parallel over batch, 2 batches per core on 8 NeuronCores.
No collectives.

v2 design (per core, per batch):
  - Loads split in chunks and issued up front; prep (PE transposes of
    Q/T into [128,1024] PSUM groups + one big f32->bf16 cast-copy per
    group on DVE) is pipelined under the main loop; batch-1 prep is
    emitted inside batch-0's qb loop so PE/DVE never idle.
  - Scores S^T[k,q] via matmul(lhsT=tT tile, rhs=qT block) in bf16.
    Exact-causal: off-diagonal k-tile pairs in [128,1024] PSUM groups;
    the 4 diagonal k-tiles are column-trimmed (512/384/256/128 cols).
  - num = max(exp(S^T/sqrt(d)), 1): exp on ScalarE (scale fused),
    max on DVE in bf16 (4x perf mode). Equals the reference's
    threshold-then-exp except on scores in (0,0.3], error <=2.7% of
    one softmax term.
  - Causal wedge: one [128,128] affine_select per diagonal k-tile on
    GpSimd (fill=0), only on the 128 columns straddling the diagonal.
  - PV + denominator fused: lhsT = num chunk [k,128q], rhs =
    [V | ones] [k,129] bf16, accumulated in PSUM over k. obanks packed
    2 subtiles per PSUM tile [128,2,129]. Denominator at column 128.
  - Normalize: obank pair copied PSUM->SBUF on DVE, then
    normalize_recip (out = pv/den) on GpSimd. Stores 1 per 2 q-blocks.
"""

from contextlib import ExitStack

import numpy as np

import concourse.bass as bass
import concourse.mybir as mybir
import concourse.tile as tile
from concourse import bacc

B, S, D = 16, 2048, 128
N_CORES = 8
B_LOC = B // N_CORES          # 2 batches per core
QB = 512                      # q-block width (matmul moving dim)
KT = 128                      # k-tile height (partition dim)
N_QB = S // QB                # 4 q-blocks
N_ST = S // 128               # 16 seq tiles
SCALE = float(1.0 / np.sqrt(D))

F32 = mybir.dt.float32
BF16 = mybir.dt.bfloat16


def build_attention_core():
    """Build the single-core SPMD graph: [B_LOC,S,D] Q/T/V -> [B_LOC,S,D] out."""
    from concourse.masks import make_identity

    nc = bacc.Bacc("TRN2", target_bir_lowering=False, debug=False,
                   num_devices=N_CORES)
    q_ext = nc.dram_tensor("Q", [B_LOC, S, D], F32, kind="ExternalInput").ap()
    t_ext = nc.dram_tensor("T", [B_LOC, S, D], F32, kind="ExternalInput").ap()
    v_ext = nc.dram_tensor("V", [B_LOC, S, D], F32, kind="ExternalInput").ap()
    o_ext = nc.dram_tensor("out", [B_LOC, S, D], F32, kind="ExternalOutput").ap()

    with tile.TileContext(nc) as tc, ExitStack() as ctx:
        nat_pool = ctx.enter_context(tc.tile_pool(name="nat", bufs=1))
        qt_pool = ctx.enter_context(tc.tile_pool(name="qt", bufs=1))
        tt_pool = ctx.enter_context(tc.tile_pool(name="tt", bufs=1))
        vb_pool = ctx.enter_context(tc.tile_pool(name="vb", bufs=1))
        num_pool = ctx.enter_context(tc.tile_pool(name="num", bufs=4))
        stg_pool = ctx.enter_context(tc.tile_pool(name="stg", bufs=2))
        fin_pool = ctx.enter_context(tc.tile_pool(name="fin", bufs=2))
        const_pool = ctx.enter_context(tc.tile_pool(name="const", bufs=1))
        # PSUM: qk pool tiles [128,1024] f32 = 2 banks x2 bufs = 4 banks;
        # out pool tiles [128,2,129] f32 = 1 bank x4 bufs = 4 banks.
        qk_psum = ctx.enter_context(tc.tile_pool(name="qk_ps", bufs=2, space="PSUM"))
        out_psum = ctx.enter_context(tc.tile_pool(name="out_ps", bufs=1, space="PSUM"))

        ident = const_pool.tile([128, 128], F32)
        make_identity(nc, ident[:])

        # nat tiles are SHARED between the two batches: batch 1's loads
        # overwrite batch 0's tiles, so the WAR dependency (b0's transposes /
        # v_aug casts must finish reading first) naturally staggers b1's DMA
        # traffic behind b0's critical chunks on the rings.
        q_nat = nat_pool.tile([128, N_ST, D], F32, name="q_nat")
        t_nat = nat_pool.tile([128, N_ST, D], F32, name="t_nat")
        v_nat = nat_pool.tile([128, N_ST, D], F32, name="v_nat")
        nats = [(q_nat, t_nat, v_nat)] * B_LOC
        # Loads are issue-parallelized across the three DMA-capable queues
        # (Sync: T, Scalar: Q, GpSimd: V) and chunked so the tiles needed by
        # q-block 0 (T/Q/V tiles 0:4) hit the DMA rings first. Batch-1 loads
        # are issued later from the GpSimd queue (staggered in the main loop)
        # so they don't steal ring bandwidth from batch 0's critical chunks.
        exts = [(t_ext, 1), (q_ext, 0), (v_ext, 2)]

        def load(b, eng, ei, t0, t1):
            ext, which = exts[ei]
            eng.dma_start(
                nats[b][which][:, t0:t1, :],
                ext[b].rearrange("(t p) d -> p t d", p=128)[:, t0:t1, :])

        for t0, t1 in ((0, 4), (4, 8), (8, 12), (12, 16)):
            load(0, nc.sync, 0, t0, t1)      # T chunks
            load(0, nc.scalar, 1, t0, t1)    # Q chunks
            load(0, nc.gpsimd, 2, t0, t1)    # V chunks

        def b1_loads():
            # batch-1 loads: all on Sync, emitted after batch 0's last nat
            # readers so the WAR deps on the shared tiles stagger them behind
            # batch 0's chunks on the DMA rings
            for t0, t1 in ((0, 8), (8, 16)):
                for ei in range(3):
                    load(1, nc.sync, ei, t0, t1)

        # ---- prep helpers ------------------------------------------------
        qTs, tTs, v_augs = {}, {}, {}

        def alloc_prep(b):
            qTs[b] = qt_pool.tile([128, N_ST, 128], BF16, name=f"qT{b}")
            tTs[b] = tt_pool.tile([128, N_ST, 128], BF16, name=f"tT{b}")
            v_augs[b] = vb_pool.tile([128, N_ST, 132], BF16, name=f"vaug{b}")

        def emit_transpose_group(b, which, t0, t1):
            """Transpose tiles [t0, t1) of Q (which=0) or T (which=1) of
            batch b into one PSUM group, then one f32->bf16 cast-copy to the
            persistent qT/tT tile."""
            src = nats[b][which]
            dst = (qTs if which == 0 else tTs)[b]
            n = t1 - t0
            ps = qk_psum.tile([128, 1024], F32, tag="qk")
            for i in range(n):
                nc.tensor.transpose(ps[:, i * 128:(i + 1) * 128],
                                    src[:, t0 + i, :], ident[:])
            nc.vector.tensor_copy(dst[:, t0:t1, :], ps[:, 0:n * 128])

        def emit_vaug(b, t0, t1):
            """Cast V tiles [t0, t1) to bf16 into v_aug (DVE)."""
            nc.vector.tensor_copy(v_augs[b][:, t0:t1, 0:D],
                                  nats[b][2][:, t0:t1, :])

        # ---- batch-0 prep: only q-block 0's needs up front; the rest is
        # interleaved between score groups via hooks so the PE FIFO never
        # queues transposes ahead of ready score matmuls. ----
        alloc_prep(0)
        alloc_prep(1)
        emit_transpose_group(0, 1, 0, 4)    # T tiles 0:4
        emit_transpose_group(0, 0, 0, 4)    # Q tiles 0:4
        emit_vaug(0, 0, 4)
        nc.gpsimd.memset(v_augs[0][:, :, D:D + 1], 1.0)

        hooks_by_point = {
            (0, 0): [
                lambda: emit_transpose_group(0, 1, 4, 8),
                lambda: (emit_transpose_group(0, 0, 4, 8),
                         emit_vaug(0, 4, 8)),
            ],
            (0, 1): [
                lambda: emit_transpose_group(0, 1, 8, 12),
                lambda: emit_transpose_group(0, 0, 8, 12),
                lambda: emit_vaug(0, 8, 12),
                lambda: emit_transpose_group(0, 1, 12, 16),
            ],
            (0, 2): [
                lambda: emit_transpose_group(0, 0, 12, 16),
                lambda: (emit_vaug(0, 12, 16), b1_loads()),
            ],
            (0, 3): [
                lambda: emit_transpose_group(1, 1, 0, 4),
                lambda: emit_transpose_group(1, 0, 0, 4),
                lambda: (emit_vaug(1, 0, 4),
                         nc.gpsimd.memset(v_augs[1][:, :, D:D + 1], 1.0)),
                lambda: emit_transpose_group(1, 1, 4, 8),
                lambda: emit_transpose_group(1, 0, 4, 8),
                lambda: emit_vaug(1, 4, 8),
            ],
            (1, 0): [
                lambda: emit_transpose_group(1, 1, 8, 16),
                lambda: emit_transpose_group(1, 0, 8, 16),
            ],
            (1, 1): [
                lambda: emit_vaug(1, 8, 16),
            ],
        }

        # ---- main attention loops ----
        def emit_qb(b, qb, fin, hooks=()):
            """Emit one q-block: scores -> exp/max/select -> PV -> normalize.
            fin: [128, 8, 128] staging tile for 2 q-blocks; this qb uses
            slot (qb % 2). hooks: thunks emitted one-per-score-group to
            interleave prep work for the next batch."""
            hooks = list(hooks)
            qT_flat = qTs[b][:].rearrange("p t q -> p (t q)")
            tT_flat = tTs[b][:].rearrange("p t k -> p (t k)")
            v_aug = v_augs[b]
            q0 = qb * QB
            c_diag = 4 * qb              # first diagonal k-tile index

            # one PSUM tile, 4 bank-aligned sub-accumulators [128, 129 of 512]
            obank = out_psum.tile([128, 4, 512], F32, tag="ob")

            def pv(c, num_ap, subs):
                """PV matmuls for k-tile c; num_ap[:, i*128:(i+1)*128] is the
                numerator chunk for sub subs[i]."""
                for i, sub in enumerate(subs):
                    nc.tensor.matmul(
                        obank[:, sub, 0:129],
                        lhsT=num_ap[:, i * 128:(i + 1) * 128],
                        rhs=v_aug[:, c, 0:129],
                        start=(c == 0),
                        stop=(c == c_diag + sub),
                    )

            # off-diagonal full k-tile pairs
            for g in range(c_diag // 2):
                cs = (2 * g, 2 * g + 1)
                s_ps = qk_psum.tile([128, 1024], F32, tag="qk")
                for j, c in enumerate(cs):
                    nc.tensor.matmul(
                        s_ps[:, j * 512:(j + 1) * 512],
                        lhsT=tT_flat[:, c * KT:(c + 1) * KT],
                        rhs=qT_flat[:, q0:q0 + QB],
                    )
                num = num_pool.tile([128, 1024], BF16)
                nc.scalar.activation(num[:], s_ps[:],
                                     mybir.ActivationFunctionType.Exp,
                                     scale=SCALE)
                nc.vector.tensor_scalar_max(num[:], num[:], 1.0)
                for j, c in enumerate(cs):
                    pv(c, num[:, j * 512:(j + 1) * 512], (0, 1, 2, 3))
                if hooks:
                    hooks.pop(0)()

            # diagonal block: k-tiles c_diag+j, trimmed to 512-128j columns,
            # packed in two PSUM groups: (j=0: 512, j=1: 384) and
            # (j=2: 256, j=3: 128).
            for grp, js in enumerate(((0, 1), (2, 3))):
                widths = [QB - 128 * j for j in js]
                s_ps = qk_psum.tile([128, 1024], F32, tag="qk")
                off = 0
                offs = []
                for j, w in zip(js, widths):
                    nc.tensor.matmul(
                        s_ps[:, off:off + w],
                        lhsT=tT_flat[:, (c_diag + j) * KT:(c_diag + j + 1) * KT],
                        rhs=qT_flat[:, q0 + 128 * j:q0 + QB],
                    )
                    offs.append(off)
                    off += w
                num = num_pool.tile([128, 1024], BF16)
                nc.scalar.activation(num[:, 0:off], s_ps[:, 0:off],
                                     mybir.ActivationFunctionType.Exp,
                                     scale=SCALE)
                nc.vector.tensor_scalar_max(num[:, 0:off], num[:, 0:off], 1.0)
                # causal wedge: first 128 computed cols of each diagonal tile
                for j, o in zip(js, offs):
                    nc.gpsimd.affine_select(
                        out=num[:, o:o + 128],
                        in_=num[:, o:o + 128],
                        compare_op=mybir.AluOpType.is_ge,
                        fill=0.0,
                        base=0,
                        channel_multiplier=-1,
                        pattern=[[1, 128]],
                    )
                for j, o, w in zip(js, offs, widths):
                    pv(c_diag + j, num[:, o:o + w], tuple(range(j, 4)))
                if hooks:
                    hooks.pop(0)()

            # ---- normalize: PSUM->SBUF copy (DVE), pv/den on GpSimd ----
            stg = stg_pool.tile([128, 4, 129], F32, tag="stg")
            nc.vector.tensor_copy(stg[:], obank[:, :, 0:129])
            for sub in range(4):
                nc.gpsimd.normalize_recip(
                    fin[:, sub, :],
                    stg[:, sub, 0:D],
                    stg[:, sub, D:D + 1],
                )

        for b in range(B_LOC):
            for qb in range(N_QB):
                fin = fin_pool.tile([128, 4, 128], F32, tag="fin")
                emit_qb(b, qb, fin, hooks_by_point.get((b, qb), ()))
                q0 = qb * QB
                nc.sync.dma_start(
                    o_ext[b, q0:q0 + QB, :].rearrange(
                        "(s p) d -> p s d", p=128),
                    fin[:])

    nc.compile()
    return nc


_NC_CACHE = None


def _get_nc():
    global _NC_CACHE
    if _NC_CACHE is None:
        _NC_CACHE = build_attention_core()
    return _NC_CACHE


def kernel(Q: np.ndarray, T: np.ndarray, V: np.ndarray) -> np.ndarray:
    """Full-input entry point: shard over batch, run 8-core SPMD, gather."""
    from concourse.bass_utils import run_bass_kernel_spmd

    Q = np.ascontiguousarray(np.asarray(Q, dtype=np.float32))
    T = np.ascontiguousarray(np.asarray(T, dtype=np.float32))
    V = np.ascontiguousarray(np.asarray(V, dtype=np.float32))
    assert Q.shape == (B, S, D), Q.shape

    nc = _get_nc()
    in_maps = [
        {
            "Q": Q[i * B_LOC:(i + 1) * B_LOC],
            "T": T[i * B_LOC:(i + 1) * B_LOC],
            "V": V[i * B_LOC:(i + 1) * B_LOC],
        }
        for i in range(N_CORES)
    ]
    res = run_bass_kernel_spmd(nc, in_maps, core_ids=list(range(N_CORES)))
    return np.concatenate([res.results[i]["out"] for i in range(N_CORES)], axis=0)


# revision 39
# speedup vs baseline: 1.1132x; 1.1132x over previous
"""Sparse-attention Trainium2 kernel (nn_AttentionLayer, B=16 S=2048 D=128).

reference semantics:
    A = Q @ T^T                     # [B,S,S]
    A = where(A > 0.3, A, 0)
    A += where(strictly_upper, -2^32, 0)
    y = softmax(A / sqrt(D)) @ V

Sharding: data-parallel over batch, 2 batches per core on 8 NeuronCores.
No collectives.

v2 design (per core, per batch):
  - Loads split in chunks and issued up front; prep (PE transposes of
    Q/T into [128,1024] PSUM groups + one big f32->bf16 cast-copy per
    group on DVE) is pipelined under the main loop; batch-1 prep is
    emitted inside batch-0's qb loop so PE/DVE never idle.
  - Scores S^T[k,q] via matmul(lhsT=tT tile, rhs=qT block) in bf16.
    Exact-causal: off-diagonal k-tile pairs in [128,1024] PSUM groups;
    the 4 diagonal k-tiles are column-trimmed (512/384/256/128 cols).
  - num = max(exp(S^T/sqrt(d)), 1): exp on ScalarE (scale fused),
    max on DVE in bf16 (4x perf mode). Equals the reference's
    threshold-then-exp except on scores in (0,0.3], error <=2.7% of
    one softmax term.
  - Causal wedge: one [128,128] affine_select per diagonal k-tile on
    GpSimd (fill=0), only on the 128 columns straddling the diagonal.
  - PV + denominator fused: lhsT = num chunk [k,128q], rhs =
    [V | ones] [k,129] bf16, accumulated in PSUM over k. obanks packed
    2 subtiles per PSUM tile [128,2,129]. Denominator at column 128.
  - Normalize: obank pair copied PSUM->SBUF on DVE, then
    normalize_recip (out = pv/den) on GpSimd. Stores 1 per 2 q-blocks.
"""

from contextlib import ExitStack

import numpy as np

import concourse.bass as bass
import concourse.mybir as mybir
import concourse.tile as tile
from concourse import bacc

B, S, D = 16, 2048, 128
N_CORES = 8
B_LOC = B // N_CORES          # 2 batches per core
QB = 512                      # q-block width (matmul moving dim)
KT = 128                      # k-tile height (partition dim)
N_QB = S // QB                # 4 q-blocks
N_ST = S // 128               # 16 seq tiles
SCALE = float(1.0 / np.sqrt(D))

F32 = mybir.dt.float32
BF16 = mybir.dt.bfloat16


def build_attention_core():
    """Build the single-core SPMD graph: [B_LOC,S,D] Q/T/V -> [B_LOC,S,D] out."""
    from concourse.masks import make_identity

    nc = bacc.Bacc("TRN2", target_bir_lowering=False, debug=False,
                   num_devices=N_CORES)
    q_ext = nc.dram_tensor("Q", [B_LOC, S, D], F32, kind="ExternalInput").ap()
    t_ext = nc.dram_tensor("T", [B_LOC, S, D], F32, kind="ExternalInput").ap()
    v_ext = nc.dram_tensor("V", [B_LOC, S, D], F32, kind="ExternalInput").ap()
    o_ext = nc.dram_tensor("out", [B_LOC, S, D], F32, kind="ExternalOutput").ap()

    with tile.TileContext(nc) as tc, ExitStack() as ctx:
        nat_pool = ctx.enter_context(tc.tile_pool(name="nat", bufs=1))
        qt_pool = ctx.enter_context(tc.tile_pool(name="qt", bufs=1))
        tt_pool = ctx.enter_context(tc.tile_pool(name="tt", bufs=1))
        vb_pool = ctx.enter_context(tc.tile_pool(name="vb", bufs=1))
        num_pool = ctx.enter_context(tc.tile_pool(name="num", bufs=6))
        stg_pool = ctx.enter_context(tc.tile_pool(name="stg", bufs=2))
        fin_pool = ctx.enter_context(tc.tile_pool(name="fin", bufs=2))
        const_pool = ctx.enter_context(tc.tile_pool(name="const", bufs=1))
        # PSUM: qk pool tiles [128,1024] f32 = 2 banks x2 bufs = 4 banks;
        # out pool tiles [128,2,129] f32 = 1 bank x4 bufs = 4 banks.
        qk_psum = ctx.enter_context(tc.tile_pool(name="qk_ps", bufs=2, space="PSUM"))
        out_psum = ctx.enter_context(tc.tile_pool(name="out_ps", bufs=1, space="PSUM"))

        ident = const_pool.tile([128, 128], F32)
        make_identity(nc, ident[:])
        # lower-triangle (incl. diagonal, q >= k) bf16 mask for the causal
        # wedge of diagonal k-tiles; applied as a DVE multiply so the Pool
        # queue never gates the PE's diagonal PV matmuls
        trimask = const_pool.tile([128, 128], BF16)
        nc.gpsimd.memset(trimask[:], 1.0)
        nc.gpsimd.affine_select(
            out=trimask[:], in_=trimask[:],
            compare_op=mybir.AluOpType.is_ge, fill=0.0,
            base=0, channel_multiplier=-1, pattern=[[1, 128]])

        # nat tiles are SHARED between the two batches: batch 1's loads
        # overwrite batch 0's tiles, so the WAR dependency (b0's transposes /
        # v_aug casts must finish reading first) naturally staggers b1's DMA
        # traffic behind b0's critical chunks on the rings.
        q_nat = nat_pool.tile([128, N_ST, D], F32, name="q_nat")
        t_nat = nat_pool.tile([128, N_ST, D], F32, name="t_nat")
        v_nat = nat_pool.tile([128, N_ST, D], F32, name="v_nat")
        nats = [(q_nat, t_nat, v_nat)] * B_LOC
        # Loads are issue-parallelized across the three DMA-capable queues
        # (Sync: T, Scalar: Q, GpSimd: V) and chunked so the tiles needed by
        # q-block 0 (T/Q/V tiles 0:4) hit the DMA rings first. Batch-1 loads
        # are issued later from the GpSimd queue (staggered in the main loop)
        # so they don't steal ring bandwidth from batch 0's critical chunks.
        exts = [(t_ext, 1), (q_ext, 0), (v_ext, 2)]

        def load(b, eng, ei, t0, t1):
            ext, which = exts[ei]
            eng.dma_start(
                nats[b][which][:, t0:t1, :],
                ext[b].rearrange("(t p) d -> p t d", p=128)[:, t0:t1, :])

        for t0, t1 in ((0, 4), (4, 8), (8, 12), (12, 16)):
            load(0, nc.sync, 0, t0, t1)      # T chunks
            load(0, nc.scalar, 1, t0, t1)    # Q chunks
            load(0, nc.gpsimd, 2, t0, t1)    # V chunks

        def b1_loads():
            # batch-1 loads: all on Sync, emitted after batch 0's last nat
            # readers so the WAR deps on the shared tiles stagger them behind
            # batch 0's chunks on the DMA rings
            for t0, t1 in ((0, 8), (8, 16)):
                for ei in range(3):
                    load(1, nc.sync, ei, t0, t1)

        # ---- prep helpers ------------------------------------------------
        qTs, tTs, v_augs = {}, {}, {}

        def alloc_prep(b):
            qTs[b] = qt_pool.tile([128, N_ST, 128], BF16, name=f"qT{b}")
            tTs[b] = tt_pool.tile([128, N_ST, 128], BF16, name=f"tT{b}")
            v_augs[b] = vb_pool.tile([128, N_ST, 132], BF16, name=f"vaug{b}")

        def emit_transpose_group(b, which, t0, t1):
            """Transpose tiles [t0, t1) of Q (which=0) or T (which=1) of
            batch b into one PSUM group, then one f32->bf16 cast-copy to the
            persistent qT/tT tile."""
            src = nats[b][which]
            dst = (qTs if which == 0 else tTs)[b]
            n = t1 - t0
            ps = qk_psum.tile([128, 1024], F32, tag="qk")
            for i in range(n):
                nc.tensor.transpose(ps[:, i * 128:(i + 1) * 128],
                                    src[:, t0 + i, :], ident[:])
            nc.vector.tensor_copy(dst[:, t0:t1, :], ps[:, 0:n * 128])

        def emit_vaug(b, t0, t1):
            """Cast V tiles [t0, t1) to bf16 into v_aug (DVE)."""
            nc.vector.tensor_copy(v_augs[b][:, t0:t1, 0:D],
                                  nats[b][2][:, t0:t1, :])

        # ---- batch-0 prep: only q-block 0's needs up front; the rest is
        # interleaved between score groups via hooks so the PE FIFO never
        # queues transposes ahead of ready score matmuls. ----
        alloc_prep(0)
        alloc_prep(1)
        emit_transpose_group(0, 1, 0, 4)    # T tiles 0:4
        emit_transpose_group(0, 0, 0, 4)    # Q tiles 0:4
        emit_vaug(0, 0, 4)
        nc.gpsimd.memset(v_augs[0][:, :, D:D + 1], 1.0)

        hooks_by_point = {
            (0, 0): [
                lambda: emit_transpose_group(0, 1, 4, 8),
                lambda: (emit_transpose_group(0, 0, 4, 8),
                         emit_vaug(0, 4, 8)),
            ],
            (0, 1): [
                lambda: emit_transpose_group(0, 1, 8, 12),
                lambda: emit_transpose_group(0, 0, 8, 12),
                lambda: emit_vaug(0, 8, 12),
                lambda: emit_transpose_group(0, 1, 12, 16),
            ],
            (0, 2): [
                lambda: emit_transpose_group(0, 0, 12, 16),
                lambda: (emit_vaug(0, 12, 16), b1_loads()),
            ],
            (0, 3): [
                lambda: emit_transpose_group(1, 1, 0, 4),
                lambda: emit_transpose_group(1, 0, 0, 4),
                lambda: (emit_vaug(1, 0, 4),
                         nc.gpsimd.memset(v_augs[1][:, :, D:D + 1], 1.0)),
                lambda: emit_transpose_group(1, 1, 4, 8),
                lambda: emit_transpose_group(1, 0, 4, 8),
                lambda: emit_vaug(1, 4, 8),
            ],
            (1, 0): [
                lambda: emit_transpose_group(1, 1, 8, 16),
                lambda: emit_transpose_group(1, 0, 8, 16),
            ],
            (1, 1): [
                lambda: emit_vaug(1, 8, 16),
            ],
        }

        # ---- main attention loops ----
        def emit_qb(b, qb, fin, hooks=()):
            """Emit one q-block: scores -> exp/max/select -> PV -> normalize.
            fin: [128, 8, 128] staging tile for 2 q-blocks; this qb uses
            slot (qb % 2). hooks: thunks emitted one-per-score-group to
            interleave prep work for the next batch."""
            hooks = list(hooks)
            qT_flat = qTs[b][:].rearrange("p t q -> p (t q)")
            tT_flat = tTs[b][:].rearrange("p t k -> p (t k)")
            v_aug = v_augs[b]
            q0 = qb * QB
            c_diag = 4 * qb              # first diagonal k-tile index

            # one PSUM tile, 4 bank-aligned sub-accumulators [128, 129 of 512]
            obank = out_psum.tile([128, 4, 512], F32, tag="ob")

            def pv(c, num_ap, subs):
                """PV matmuls for k-tile c; num_ap[:, i*128:(i+1)*128] is the
                numerator chunk for sub subs[i]."""
                for i, sub in enumerate(subs):
                    nc.tensor.matmul(
                        obank[:, sub, 0:129],
                        lhsT=num_ap[:, i * 128:(i + 1) * 128],
                        rhs=v_aug[:, c, 0:129],
                        start=(c == 0),
                        stop=(c == c_diag + sub),
                    )

            # off-diagonal full k-tile pairs
            for g in range(c_diag // 2):
                cs = (2 * g, 2 * g + 1)
                s_ps = qk_psum.tile([128, 1024], F32, tag="qk")
                for j, c in enumerate(cs):
                    nc.tensor.matmul(
                        s_ps[:, j * 512:(j + 1) * 512],
                        lhsT=tT_flat[:, c * KT:(c + 1) * KT],
                        rhs=qT_flat[:, q0:q0 + QB],
                    )
                num = num_pool.tile([128, 1024], BF16)
                nc.scalar.activation(num[:], s_ps[:],
                                     mybir.ActivationFunctionType.Exp,
                                     scale=SCALE)
                nc.vector.tensor_scalar_max(num[:], num[:], 1.0)
                for j, c in enumerate(cs):
                    pv(c, num[:, j * 512:(j + 1) * 512], (0, 1, 2, 3))
                if hooks:
                    hooks.pop(0)()

            # diagonal block: k-tiles c_diag+j, trimmed to 512-128j columns,
            # packed in two PSUM groups: (j=0: 512, j=1: 384) and
            # (j=2: 256, j=3: 128).
            for grp, js in enumerate(((0, 1), (2, 3))):
                widths = [QB - 128 * j for j in js]
                s_ps = qk_psum.tile([128, 1024], F32, tag="qk")
                off = 0
                offs = []
                for j, w in zip(js, widths):
                    nc.tensor.matmul(
                        s_ps[:, off:off + w],
                        lhsT=tT_flat[:, (c_diag + j) * KT:(c_diag + j + 1) * KT],
                        rhs=qT_flat[:, q0 + 128 * j:q0 + QB],
                    )
                    offs.append(off)
                    off += w
                num = num_pool.tile([128, 1024], BF16)
                nc.scalar.activation(num[:, 0:off], s_ps[:, 0:off],
                                     mybir.ActivationFunctionType.Exp,
                                     scale=SCALE)
                nc.vector.tensor_scalar_max(num[:, 0:off], num[:, 0:off], 1.0)
                # causal wedge: first 128 computed cols of each diagonal tile
                for j, o in zip(js, offs):
                    nc.vector.tensor_mul(num[:, o:o + 128],
                                         num[:, o:o + 128], trimask[:])
                for j, o, w in zip(js, offs, widths):
                    pv(c_diag + j, num[:, o:o + w], tuple(range(j, 4)))
                if hooks:
                    hooks.pop(0)()

            # ---- normalize: PSUM->SBUF copy (DVE), pv/den on GpSimd ----
            stg = stg_pool.tile([128, 4, 129], F32, tag="stg")
            nc.vector.tensor_copy(stg[:], obank[:, :, 0:129])
            for sub in range(4):
                nc.gpsimd.normalize_recip(
                    fin[:, sub, :],
                    stg[:, sub, 0:D],
                    stg[:, sub, D:D + 1],
                )

        for b in range(B_LOC):
            for qb in range(N_QB):
                fin = fin_pool.tile([128, 4, 128], F32, tag="fin")
                emit_qb(b, qb, fin, hooks_by_point.get((b, qb), ()))
                q0 = qb * QB
                nc.sync.dma_start(
                    o_ext[b, q0:q0 + QB, :].rearrange(
                        "(s p) d -> p s d", p=128),
                    fin[:])

    nc.compile()
    return nc


_NC_CACHE = None


def _get_nc():
    global _NC_CACHE
    if _NC_CACHE is None:
        _NC_CACHE = build_attention_core()
    return _NC_CACHE


def kernel(Q: np.ndarray, T: np.ndarray, V: np.ndarray) -> np.ndarray:
    """Full-input entry point: shard over batch, run 8-core SPMD, gather."""
    from concourse.bass_utils import run_bass_kernel_spmd

    Q = np.ascontiguousarray(np.asarray(Q, dtype=np.float32))
    T = np.ascontiguousarray(np.asarray(T, dtype=np.float32))
    V = np.ascontiguousarray(np.asarray(V, dtype=np.float32))
    assert Q.shape == (B, S, D), Q.shape

    nc = _get_nc()
    in_maps = [
        {
            "Q": Q[i * B_LOC:(i + 1) * B_LOC],
            "T": T[i * B_LOC:(i + 1) * B_LOC],
            "V": V[i * B_LOC:(i + 1) * B_LOC],
        }
        for i in range(N_CORES)
    ]
    res = run_bass_kernel_spmd(nc, in_maps, core_ids=list(range(N_CORES)))
    return np.concatenate([res.results[i]["out"] for i in range(N_CORES)], axis=0)


# revision 44
# speedup vs baseline: 1.1136x; 1.0003x over previous
"""Sparse-attention Trainium2 kernel (nn_AttentionLayer, B=16 S=2048 D=128).

reference semantics:
    A = Q @ T^T                     # [B,S,S]
    A = where(A > 0.3, A, 0)
    A += where(strictly_upper, -2^32, 0)
    y = softmax(A / sqrt(D)) @ V

Sharding: data-parallel over batch, 2 batches per core on 8 NeuronCores.
No collectives.

v2 design (per core, per batch):
  - Loads split in chunks and issued up front; prep (PE transposes of
    Q/T into [128,1024] PSUM groups + one big f32->bf16 cast-copy per
    group on DVE) is pipelined under the main loop; batch-1 prep is
    emitted inside batch-0's qb loop so PE/DVE never idle.
  - Scores S^T[k,q] via matmul(lhsT=tT tile, rhs=qT block) in bf16.
    Exact-causal: off-diagonal k-tile pairs in [128,1024] PSUM groups;
    the 4 diagonal k-tiles are column-trimmed (512/384/256/128 cols).
  - num = max(exp(S^T/sqrt(d)), 1): exp on ScalarE (scale fused),
    max on DVE in bf16 (4x perf mode). Equals the reference's
    threshold-then-exp except on scores in (0,0.3], error <=2.7% of
    one softmax term.
  - Causal wedge: one [128,128] affine_select per diagonal k-tile on
    GpSimd (fill=0), only on the 128 columns straddling the diagonal.
  - PV + denominator fused: lhsT = num chunk [k,128q], rhs =
    [V | ones] [k,129] bf16, accumulated in PSUM over k. obanks packed
    2 subtiles per PSUM tile [128,2,129]. Denominator at column 128.
  - Normalize: obank pair copied PSUM->SBUF on DVE, then
    normalize_recip (out = pv/den) on GpSimd. Stores 1 per 2 q-blocks.
"""

from contextlib import ExitStack

import numpy as np

import concourse.bass as bass
import concourse.mybir as mybir
import concourse.tile as tile
from concourse import bacc

B, S, D = 16, 2048, 128
N_CORES = 8
B_LOC = B // N_CORES          # 2 batches per core
QB = 512                      # q-block width (matmul moving dim)
KT = 128                      # k-tile height (partition dim)
N_QB = S // QB                # 4 q-blocks
N_ST = S // 128               # 16 seq tiles
SCALE = float(1.0 / np.sqrt(D))

F32 = mybir.dt.float32
BF16 = mybir.dt.bfloat16


def build_attention_core():
    """Build the single-core SPMD graph: [B_LOC,S,D] Q/T/V -> [B_LOC,S,D] out."""
    from concourse.masks import make_identity

    nc = bacc.Bacc("TRN2", target_bir_lowering=False, debug=False,
                   num_devices=N_CORES)
    q_ext = nc.dram_tensor("Q", [B_LOC, S, D], F32, kind="ExternalInput").ap()
    t_ext = nc.dram_tensor("T", [B_LOC, S, D], F32, kind="ExternalInput").ap()
    v_ext = nc.dram_tensor("V", [B_LOC, S, D], F32, kind="ExternalInput").ap()
    o_ext = nc.dram_tensor("out", [B_LOC, S, D], F32, kind="ExternalOutput").ap()

    with tile.TileContext(nc) as tc, ExitStack() as ctx:
        nat_pool = ctx.enter_context(tc.tile_pool(name="nat", bufs=1))
        qt_pool = ctx.enter_context(tc.tile_pool(name="qt", bufs=1))
        tt_pool = ctx.enter_context(tc.tile_pool(name="tt", bufs=1))
        vb_pool = ctx.enter_context(tc.tile_pool(name="vb", bufs=1))
        num_pool = ctx.enter_context(tc.tile_pool(name="num", bufs=6))
        stg_pool = ctx.enter_context(tc.tile_pool(name="stg", bufs=2))
        fin_pool = ctx.enter_context(tc.tile_pool(name="fin", bufs=2))
        const_pool = ctx.enter_context(tc.tile_pool(name="const", bufs=1))
        # PSUM: qk pool tiles [128,1024] f32 = 2 banks x2 bufs = 4 banks;
        # out pool tiles [128,2,129] f32 = 1 bank x4 bufs = 4 banks.
        qk_psum = ctx.enter_context(tc.tile_pool(name="qk_ps", bufs=2, space="PSUM"))
        out_psum = ctx.enter_context(tc.tile_pool(name="out_ps", bufs=1, space="PSUM"))

        ident = const_pool.tile([128, 128], F32)
        make_identity(nc, ident[:])
        # lower-triangle (incl. diagonal, q >= k) bf16 mask for the causal
        # wedge of diagonal k-tiles; applied as a DVE multiply so the Pool
        # queue never gates the PE's diagonal PV matmuls
        trimask = const_pool.tile([128, 128], BF16)
        nc.gpsimd.memset(trimask[:], 1.0)
        nc.gpsimd.affine_select(
            out=trimask[:], in_=trimask[:],
            compare_op=mybir.AluOpType.is_ge, fill=0.0,
            base=0, channel_multiplier=-1, pattern=[[1, 128]])

        # nat tiles are SHARED between the two batches: batch 1's loads
        # overwrite batch 0's tiles, so the WAR dependency (b0's transposes /
        # v_aug casts must finish reading first) naturally staggers b1's DMA
        # traffic behind b0's critical chunks on the rings.
        q_nat = nat_pool.tile([128, N_ST, D], F32, name="q_nat")
        t_nat = nat_pool.tile([128, N_ST, D], F32, name="t_nat")
        v_nat = nat_pool.tile([128, N_ST, D], F32, name="v_nat")
        nats = [(q_nat, t_nat, v_nat)] * B_LOC
        # Loads are issue-parallelized across the three DMA-capable queues
        # (Sync: T, Scalar: Q, GpSimd: V) and chunked so the tiles needed by
        # q-block 0 (T/Q/V tiles 0:4) hit the DMA rings first. Batch-1 loads
        # are issued later from the GpSimd queue (staggered in the main loop)
        # so they don't steal ring bandwidth from batch 0's critical chunks.
        exts = [(t_ext, 1), (q_ext, 0), (v_ext, 2)]

        def load(b, eng, ei, t0, t1):
            ext, which = exts[ei]
            eng.dma_start(
                nats[b][which][:, t0:t1, :],
                ext[b].rearrange("(t p) d -> p t d", p=128)[:, t0:t1, :])

        # First wave: only what q-block 0's first score group needs (T tiles
        # 0:2, Q tiles 0:4) so it clears the DMA rings fastest.
        load(0, nc.sync, 0, 0, 2)
        load(0, nc.scalar, 1, 0, 4)
        load(0, nc.gpsimd, 2, 0, 4)
        load(0, nc.sync, 0, 2, 4)
        for t0, t1 in ((4, 8), (8, 12), (12, 16)):
            load(0, nc.sync, 0, t0, t1)      # T chunks
            load(0, nc.scalar, 1, t0, t1)    # Q chunks
            load(0, nc.gpsimd, 2, t0, t1)    # V chunks

        def b1_loads():
            # batch-1 loads: all on Sync, emitted after batch 0's last nat
            # readers so the WAR deps on the shared tiles stagger them behind
            # batch 0's chunks on the DMA rings
            for t0, t1 in ((0, 8), (8, 16)):
                for ei in range(3):
                    load(1, nc.sync, ei, t0, t1)

        # ---- prep helpers ------------------------------------------------
        qTs, tTs, v_augs = {}, {}, {}

        def alloc_prep(b):
            qTs[b] = qt_pool.tile([128, N_ST, 128], BF16, name=f"qT{b}")
            tTs[b] = tt_pool.tile([128, N_ST, 128], BF16, name=f"tT{b}")
            v_augs[b] = vb_pool.tile([128, N_ST, 132], BF16, name=f"vaug{b}")

        def emit_transpose_group(b, which, t0, t1):
            """Transpose tiles [t0, t1) of Q (which=0) or T (which=1) of
            batch b into one PSUM group, then one f32->bf16 cast-copy to the
            persistent qT/tT tile."""
            src = nats[b][which]
            dst = (qTs if which == 0 else tTs)[b]
            n = t1 - t0
            ps = qk_psum.tile([128, 1024], F32, tag="qk")
            for i in range(n):
                nc.tensor.transpose(ps[:, i * 128:(i + 1) * 128],
                                    src[:, t0 + i, :], ident[:])
            nc.vector.tensor_copy(dst[:, t0:t1, :], ps[:, 0:n * 128])

        def emit_vaug(b, t0, t1):
            """Cast V tiles [t0, t1) to bf16 into v_aug (DVE)."""
            nc.vector.tensor_copy(v_augs[b][:, t0:t1, 0:D],
                                  nats[b][2][:, t0:t1, :])

        # ---- batch-0 prep: only q-block 0's needs up front; the rest is
        # interleaved between score groups via hooks so the PE FIFO never
        # queues transposes ahead of ready score matmuls. ----
        alloc_prep(0)
        alloc_prep(1)
        emit_transpose_group(0, 1, 0, 2)    # T tiles 0:2 — first score group
        emit_transpose_group(0, 0, 0, 4)    # Q tiles 0:4
        emit_transpose_group(0, 1, 2, 4)    # T tiles 2:4
        emit_vaug(0, 0, 4)
        nc.gpsimd.memset(v_augs[0][:, :, D:D + 1], 1.0)

        hooks_by_point = {
            (0, 0): [
                lambda: emit_transpose_group(0, 1, 4, 8),
                lambda: (emit_transpose_group(0, 0, 4, 8),
                         emit_vaug(0, 4, 8)),
            ],
            (0, 1): [
                lambda: emit_transpose_group(0, 1, 8, 12),
                lambda: emit_transpose_group(0, 0, 8, 12),
                lambda: emit_vaug(0, 8, 12),
                lambda: emit_transpose_group(0, 1, 12, 16),
            ],
            (0, 2): [
                lambda: emit_transpose_group(0, 0, 12, 16),
                lambda: (emit_vaug(0, 12, 16), b1_loads()),
            ],
            (0, 3): [
                lambda: emit_transpose_group(1, 1, 0, 4),
                lambda: emit_transpose_group(1, 0, 0, 4),
                lambda: (emit_vaug(1, 0, 4),
                         nc.gpsimd.memset(v_augs[1][:, :, D:D + 1], 1.0)),
                lambda: emit_transpose_group(1, 1, 4, 8),
                lambda: emit_transpose_group(1, 0, 4, 8),
                lambda: emit_vaug(1, 4, 8),
            ],
            (1, 0): [
                lambda: emit_transpose_group(1, 1, 8, 16),
                lambda: emit_transpose_group(1, 0, 8, 16),
            ],
            (1, 1): [
                lambda: emit_vaug(1, 8, 16),
            ],
        }

        # ---- main attention loops ----
        def emit_qb(b, qb, fin, hooks=()):
            """Emit one q-block: scores -> exp/max/select -> PV -> normalize.
            fin: [128, 8, 128] staging tile for 2 q-blocks; this qb uses
            slot (qb % 2). hooks: thunks emitted one-per-score-group to
            interleave prep work for the next batch."""
            hooks = list(hooks)
            qT_flat = qTs[b][:].rearrange("p t q -> p (t q)")
            tT_flat = tTs[b][:].rearrange("p t k -> p (t k)")
            v_aug = v_augs[b]
            q0 = qb * QB
            c_diag = 4 * qb              # first diagonal k-tile index

            # one PSUM tile, 4 bank-aligned sub-accumulators [128, 129 of 512]
            obank = out_psum.tile([128, 4, 512], F32, tag="ob")

            def pv(c, num_ap, subs):
                """PV matmuls for k-tile c; num_ap[:, i*128:(i+1)*128] is the
                numerator chunk for sub subs[i]."""
                for i, sub in enumerate(subs):
                    nc.tensor.matmul(
                        obank[:, sub, 0:129],
                        lhsT=num_ap[:, i * 128:(i + 1) * 128],
                        rhs=v_aug[:, c, 0:129],
                        start=(c == 0),
                        stop=(c == c_diag + sub),
                    )

            # off-diagonal full k-tile pairs
            for g in range(c_diag // 2):
                cs = (2 * g, 2 * g + 1)
                s_ps = qk_psum.tile([128, 1024], F32, tag="qk")
                for j, c in enumerate(cs):
                    nc.tensor.matmul(
                        s_ps[:, j * 512:(j + 1) * 512],
                        lhsT=tT_flat[:, c * KT:(c + 1) * KT],
                        rhs=qT_flat[:, q0:q0 + QB],
                    )
                num = num_pool.tile([128, 1024], BF16)
                nc.scalar.activation(num[:], s_ps[:],
                                     mybir.ActivationFunctionType.Exp,
                                     scale=SCALE)
                nc.vector.tensor_scalar_max(num[:], num[:], 1.0)
                for j, c in enumerate(cs):
                    pv(c, num[:, j * 512:(j + 1) * 512], (0, 1, 2, 3))
                if hooks:
                    hooks.pop(0)()

            # diagonal block: k-tiles c_diag+j, trimmed to 512-128j columns,
            # packed in two PSUM groups: (j=0: 512, j=1: 384) and
            # (j=2: 256, j=3: 128).
            for grp, js in enumerate(((0, 1), (2, 3))):
                widths = [QB - 128 * j for j in js]
                s_ps = qk_psum.tile([128, 1024], F32, tag="qk")
                off = 0
                offs = []
                for j, w in zip(js, widths):
                    nc.tensor.matmul(
                        s_ps[:, off:off + w],
                        lhsT=tT_flat[:, (c_diag + j) * KT:(c_diag + j + 1) * KT],
                        rhs=qT_flat[:, q0 + 128 * j:q0 + QB],
                    )
                    offs.append(off)
                    off += w
                num = num_pool.tile([128, 1024], BF16)
                nc.scalar.activation(num[:, 0:off], s_ps[:, 0:off],
                                     mybir.ActivationFunctionType.Exp,
                                     scale=SCALE)
                nc.vector.tensor_scalar_max(num[:, 0:off], num[:, 0:off], 1.0)
                # causal wedge: first 128 computed cols of each diagonal tile
                for j, o in zip(js, offs):
                    nc.vector.tensor_mul(num[:, o:o + 128],
                                         num[:, o:o + 128], trimask[:])
                for j, o, w in zip(js, offs, widths):
                    pv(c_diag + j, num[:, o:o + w], tuple(range(j, 4)))
                if hooks:
                    hooks.pop(0)()

            # ---- normalize: PSUM->SBUF copy (DVE), pv/den on GpSimd ----
            # copied in sub pairs so the first norms start before the last
            # sub's PV accumulation finishes
            stg = stg_pool.tile([128, 4, 129], F32, tag="stg")
            for p in range(2):
                nc.vector.tensor_copy(stg[:, 2 * p:2 * p + 2, :],
                                      obank[:, 2 * p:2 * p + 2, 0:129])
                for s in range(2):
                    sub = 2 * p + s
                    nc.gpsimd.normalize_recip(
                        fin[:, sub, :],
                        stg[:, sub, 0:D],
                        stg[:, sub, D:D + 1],
                    )

        for b in range(B_LOC):
            for qb in range(N_QB):
                fin = fin_pool.tile([128, 4, 128], F32, tag="fin")
                emit_qb(b, qb, fin, hooks_by_point.get((b, qb), ()))
                q0 = qb * QB
                nc.sync.dma_start(
                    o_ext[b, q0:q0 + QB, :].rearrange(
                        "(s p) d -> p s d", p=128),
                    fin[:])

    nc.compile()
    return nc


_NC_CACHE = None


def _get_nc():
    global _NC_CACHE
    if _NC_CACHE is None:
        _NC_CACHE = build_attention_core()
    return _NC_CACHE


def kernel(Q: np.ndarray, T: np.ndarray, V: np.ndarray) -> np.ndarray:
    """Full-input entry point: shard over batch, run 8-core SPMD, gather."""
    from concourse.bass_utils import run_bass_kernel_spmd

    Q = np.ascontiguousarray(np.asarray(Q, dtype=np.float32))
    T = np.ascontiguousarray(np.asarray(T, dtype=np.float32))
    V = np.ascontiguousarray(np.asarray(V, dtype=np.float32))
    assert Q.shape == (B, S, D), Q.shape

    nc = _get_nc()
    in_maps = [
        {
            "Q": Q[i * B_LOC:(i + 1) * B_LOC],
            "T": T[i * B_LOC:(i + 1) * B_LOC],
            "V": V[i * B_LOC:(i + 1) * B_LOC],
        }
        for i in range(N_CORES)
    ]
    res = run_bass_kernel_spmd(nc, in_maps, core_ids=list(range(N_CORES)))
    return np.concatenate([res.results[i]["out"] for i in range(N_CORES)], axis=0)
